# revision 14
# baseline (speedup 1.0000x reference)
"""AttentionBlock (1x1-conv QKV + 4-head softmax attention + 1x1-conv proj)
on 8 Trainium2 NeuronCores.

Sharding: data-parallel over (batch b, query-half h) -> 8 shards. Each core
gets x rotated so its 2048 query columns are always columns 0:2048 (key order
is a permutation, which softmax-attention is invariant to), computes
qkv projections, 4-head attention for its half of the queries, and the output
projection for its [256, 2048] output slice. No collectives.

Core kernel tricks:
  - all matmuls in float32r (full-rate PE, ~1.5e-4 rel rounding)
  - S^T = K^T Q with two heads row-tiled in the PE array (K=64 each)
  - exp of scores: half the heads on the Scalar engine (exact), half via a
    fused custom DVE op ((x+c0)((x+c1)x+c2))^8 ~ C*e^x (scale cancels in
    softmax; assignment is per-(head, query-tile) so rows stay consistent)
  - attn @ V with two heads col-tiled, plus 4-way col-tiled ones-matmul
    rowsums accumulated in PSUM
  - softmax normalization via reciprocal + a tiny broadcast matmul
"""
import sys

sys.path.insert(0, '/opt/trn_rl_repo')

import numpy as np
from contextlib import ExitStack

from concourse import bass, bacc, mybir
import concourse.tile as tile
from concourse import dve_ops
from concourse.dve_ops import DveOp, OPS, CUSTOM_DVE_SPECS, _SUB_OPCODE_FOR_NAME
from concourse.dve_spec import Spec, Src0, Src1, C0, C1, C2, C3, lower, sq, _spill_c3_to_src1
from concourse.dve_uop import DveOpSpec
from concourse.bass_utils import run_bass_kernel_spmd

F32 = mybir.dt.float32
F32R = mybir.dt.float32r
BF16 = mybir.dt.bfloat16
ActFn = mybir.ActivationFunctionType

B, C, H, W = 4, 256, 64, 64
HEADS, DH = 4, 64
N = H * W            # 4096 keys
NQ = N // 2          # 2048 queries per core
NT = 512             # query tile (one PSUM bank of fp32)
N_NT = NQ // NT      # 4 query tiles
N_MC = N // 128      # 32 key chunks

# exp(x) ~ C * [q3(x) * (x^2 + b0 x + b1)]^16 over x in [-8.8, 8.4]
# (max rel err 3.3e-4; the constant C cancels in softmax normalization).
# Two DVE instructions: EXP5A computes the cubic q3, EXP5B multiplies by the
# monic quadratic and raises to the 16th power.
EXP_A = (0.00039684202121525346, 2.589769573122113e-05,
         6.891462469732395e-07, 7.771052073346383e-09)   # a0..a3
EXP_B = (-6.95331830849084, 2519.7822812996437)          # b0, b1


def _ref_exp5a(in0, in1, c0, c1, c2):
    x = in0.astype(np.float32)
    a3 = in1.astype(np.float32) if isinstance(in1, np.ndarray) else np.float32(in1)
    return (((a3 * x + np.float32(c2)) * x + np.float32(c1)) * x
            + np.float32(c0)).astype(np.float32)


def _ref_exp5b(in0, in1, c0, c1, c2):
    x = in0.astype(np.float32)
    q3 = in1.astype(np.float32)
    p = (q3 * ((x + np.float32(c0)) * x + np.float32(c1))).astype(np.float32)
    for _ in range(4):
        p = (p * p).astype(np.float32)
    return p


def _register(name, spec, rd1_en):
    row = dve_ops._CUSTOM_DVE_ROW_BASE + len(OPS)
    assert row < 0x20
    _SUB_OPCODE_FOR_NAME[name] = row
    shas = {}
    for ver in ("v3", "v4"):
        uops = lower(spec, ver=ver)
        shas[ver] = DveOpSpec(name=name, opcode=row, uops=uops, rd1_en=rd1_en).sha(ver)
    op = DveOp(name, spec, subdim=False, uops_sha=shas)
    OPS.append(op)
    CUSTOM_DVE_SPECS[name] = spec
    return op


def register_exp_op():
    if "EXP5A_ANT" in _SUB_OPCODE_FOR_NAME:
        a = next(op for op in OPS if op.name == "EXP5A_ANT")
        b = next(op for op in OPS if op.name == "EXP5B_ANT")
        return a, b
    x = Src0
    body_a = _spill_c3_to_src1(((C3 * x + C2) * x + C1) * x + C0)
    op_a = _register("EXP5A_ANT", Spec(body=body_a, reference=_ref_exp5a), True)
    body_b = sq(sq(sq(sq(Src1 * ((x + C0) * x + C1)))))
    op_b = _register("EXP5B_ANT", Spec(body=body_b, reference=_ref_exp5b), True)
    return op_a, op_b


def emit_exp_dve(nc, ops, out, in_, y1, a3_t):
    op_a, op_b = ops
    nc.vector._custom_dve(op_a, out=y1, in0=in_, in1=a3_t,
                          s0=float(EXP_A[0]), s1=float(EXP_A[1]), imm2=float(EXP_A[2]))
    return nc.vector._custom_dve(op_b, out=out, in0=in_, in1=y1,
                                 s0=float(EXP_B[0]), s1=float(EXP_B[1]))


# exp-engine assignment per (head-pair, query-tile): True -> ACT, False -> DVE.
# Must be constant per (pair, nt) so each softmax row uses one implementation.
# EXP_ACT_UNITS = how many of the 8 (pair, nt) units run on ACT (rest on DVE).
import os as _os
_ACT_UNITS = int(_os.environ.get("EXP_ACT_UNITS", "6"))
_order = [(pair, nt) for nt in range(N_NT) for pair in (0, 1)]
EXP_ON_ACT = {u: (i < _ACT_UNITS) for i, u in enumerate(_order)}


def build_program(exp_op):
    nc = bacc.Bacc(target_bir_lowering=False)

    x_d = nc.declare_dram_parameter("x", [C, N], F32, isOutput=False)
    wq_d = nc.declare_dram_parameter("wq", [C, C], F32, isOutput=False)
    wk_d = nc.declare_dram_parameter("wk", [C, C], F32, isOutput=False)
    wv_d = nc.declare_dram_parameter("wv", [C, C], F32, isOutput=False)
    wp_d = nc.declare_dram_parameter("wp", [C, C], F32, isOutput=False)
    bias_d = nc.declare_dram_parameter("bias", [128, 2], F32, isOutput=False)
    y_d = nc.declare_dram_parameter("y", [C, NQ], F32, isOutput=True)
    import os as _os
    _DBG = bool(int(_os.environ.get("KERNEL_DEBUG", "0")))
    dbg = {}
    if _DBG:
        BF16_ = mybir.dt.bfloat16
        dbg["q0"] = nc.declare_dram_parameter("dbg_q0", [128, 512], F32, isOutput=True)
        dbg["st0"] = nc.declare_dram_parameter("dbg_st0", [128, 1024], F32, isOutput=True)
        dbg["e0"] = nc.declare_dram_parameter("dbg_e0", [128, 1024], BF16_, isOutput=True)
        dbg["e1"] = nc.declare_dram_parameter("dbg_e1", [128, 1024], BF16_, isOutput=True)
        dbg["rs"] = nc.declare_dram_parameter("dbg_rs", [128, 512], F32, isOutput=True)
        dbg["rsinv"] = nc.declare_dram_parameter("dbg_rsinv", [128, 512], F32, isOutput=True)
        dbg["rb0"] = nc.declare_dram_parameter("dbg_rb0", [128, 512], F32, isOutput=True)
        dbg["rb1"] = nc.declare_dram_parameter("dbg_rb1", [128, 512], F32, isOutput=True)
        dbg["po0"] = nc.declare_dram_parameter("dbg_po0", [128, 512], F32, isOutput=True)
        dbg["vt0"] = nc.declare_dram_parameter("dbg_vt0", [128, 256], BF16_, isOutput=True)

    with tile.TileContext(nc) as tc, ExitStack() as ctx:
        sb = ctx.enter_context(tc.tile_pool(name="sb", bufs=1))
        pex = ctx.enter_context(tc.tile_pool(name="pex", bufs=3))
        pout = ctx.enter_context(tc.tile_pool(name="pout", bufs=2))
        ps = ctx.enter_context(tc.tile_pool(name="ps", bufs=1, space="PSUM"))

        # ---------------- load + round inputs to f32r ----------------
        x_f = [sb.tile([128, N], F32, tag=f"xf{i}", name=f"xf{i}") for i in range(2)]
        x_r = [sb.tile([128, N], F32R, tag=f"xr{i}", name=f"xr{i}") for i in range(2)]
        for kc in range(2):
            nc.sync.dma_start(out=x_f[kc], in_=x_d[kc * 128:(kc + 1) * 128, :])
        nc.scalar.copy(x_r[0][:, :], x_f[0][:, :])
        nc.vector.tensor_copy(x_r[1][:, :], x_f[1][:, :])

        w_sb = {}
        for name, dram in (("wq", wq_d), ("wk", wk_d), ("wv", wv_d), ("wp", wp_d)):
            tiles = []
            for kc in range(2):
                f = sb.tile([128, C], F32, tag="wf", name=f"{name}f{kc}")
                nc.sync.dma_start(out=f, in_=dram[kc * 128:(kc + 1) * 128, :])
                r = sb.tile([128, C], F32R, tag=f"{name}{kc}", name=f"{name}r{kc}")
                (nc.vector.tensor_copy if kc else nc.scalar.copy)(r[:, :], f[:, :])
                tiles.append(r)
            w_sb[name] = tiles
        bias_sb = sb.tile([128, 2], F32, tag="bias")
        nc.sync.dma_start(out=bias_sb, in_=bias_d[:, :])

        # constants: ones column + broadcast matrices
        ones = sb.tile([128, 1], BF16, tag="ones")
        nc.vector.memset(ones, 1.0)
        a3_t = sb.tile([128, 1], F32, tag="a3")
        nc.vector.memset(a3_t, float(EXP_A[3]))
        # broadcast matrix: out[m, n] = rhs[32*head(m) + 64*oc, n]
        bc_f = sb.tile([128, 256], F32, tag="bc_f")
        nc.vector.memset(bc_f, 0.0)
        nc.vector.memset(bc_f[0:1, 0:64], 1.0)
        nc.vector.memset(bc_f[32:33, 64:128], 1.0)
        nc.vector.memset(bc_f[64:65, 128:192], 1.0)
        nc.vector.memset(bc_f[96:97, 192:256], 1.0)
        bc = sb.tile([128, 256], F32R, tag="bc")
        nc.vector.tensor_copy(bc, bc_f[:, :])


        # ---------------- phase 1: qkv projections ----------------
        q_sb = [sb.tile([128, NQ], F32R, tag=f"q{oc}", name=f"q_sb{oc}") for oc in range(2)]
        k_sb = [sb.tile([128, N], F32R, tag=f"k{oc}", name=f"k_sb{oc}") for oc in range(2)]
        vT_sb = sb.tile([128, N_MC * 256], BF16, tag="vT")

        for oc in range(2):
            for nt in range(N_NT):
                pq = ps.tile([128, 512], F32, tag="s", bufs=2, name=f"pq{oc}_{nt}")
                sl = slice(nt * 512, (nt + 1) * 512)
                nc.tensor.matmul(out=pq[:, :], lhsT=w_sb["wq"][0][:, oc * 128:(oc + 1) * 128],
                                 rhs=x_r[0][:, sl], start=True, stop=False)
                nc.tensor.matmul(out=pq[:, :], lhsT=w_sb["wq"][1][:, oc * 128:(oc + 1) * 128],
                                 rhs=x_r[1][:, sl], start=False, stop=True)
                nc.vector.tensor_copy(q_sb[oc][:, sl], pq[:, :])
                if _DBG and oc == 0 and nt == 0:
                    nc.sync.dma_start(out=dbg["q0"][:, :], in_=q_sb[0][:, 0:512].bitcast(F32))
        for oc in range(2):
            for nt in range(2 * N_NT):
                pk = ps.tile([128, 512], F32, tag="s", bufs=2, name=f"pk{oc}_{nt}")
                sl = slice(nt * 512, (nt + 1) * 512)
                nc.tensor.matmul(out=pk[:, :], lhsT=w_sb["wk"][0][:, oc * 128:(oc + 1) * 128],
                                 rhs=x_r[0][:, sl], start=True, stop=False)
                nc.tensor.matmul(out=pk[:, :], lhsT=w_sb["wk"][1][:, oc * 128:(oc + 1) * 128],
                                 rhs=x_r[1][:, sl], start=False, stop=True)
                (nc.vector.tensor_copy if nt % 2 else nc.scalar.copy)(k_sb[oc][:, sl], pk[:, :])
        for mc in range(N_MC):
            pv = ps.tile([128, 256], F32, tag="rs", name=f"pv{mc}")
            msl = slice(mc * 128, (mc + 1) * 128)
            nc.tensor.matmul(out=pv[:, :], lhsT=x_r[0][:, msl], rhs=w_sb["wv"][0][:, :],
                             start=True, stop=False)
            nc.tensor.matmul(out=pv[:, :], lhsT=x_r[1][:, msl], rhs=w_sb["wv"][1][:, :],
                             start=False, stop=True)
            nc.scalar.copy(vT_sb[:, mc * 256:(mc + 1) * 256], pv[:, :])
            if _DBG and mc == 0:
                nc.sync.dma_start(out=dbg["vt0"][:, :], in_=vT_sb[:, 0:256])

        import os
        _PH = int(os.environ.get("KERNEL_PHASES", "3"))
        # ---------------- phase 2: attention ----------------
        out_sp = [sb.tile([128, NQ], F32R, tag=f"osp{oc}", name=f"out_sp{oc}") for oc in range(2)]

        if _PH < 2:
            zero_f = sb.tile([128, 512], F32, tag="zero_f")
            nc.vector.memset(zero_f, 0.0)
            for oc in range(2):
                for z in range(4):
                    nc.vector.tensor_copy(out_sp[oc][:, z * 512:(z + 1) * 512], zero_f[:, :])
        for nt in range(N_NT if _PH >= 2 else 0):
            qsl = slice(nt * 512, (nt + 1) * 512)
            po = [ps.tile([128, 512], F32, tag=f"o{pair}", name=f"po{pair}_{nt}") for pair in (0, 1)]
            prs = ps.tile([128, 512], F32, tag="rs", name=f"prs{nt}")
            for mc in range(N_MC):
                msl = slice(mc * 128, (mc + 1) * 128)
                exps = []
                for pair in (0, 1):
                    pst = ps.tile([128, 1024], F32, tag="s", bufs=2, name=f"pst{pair}_{nt}_{mc}")
                    # S^T: two heads row-tiled (dh=64 each)
                    nc.tensor.matmul(out=pst[:, 0:512],
                                     lhsT=k_sb[pair][0:64, msl], rhs=q_sb[pair][0:64, qsl],
                                     start=True, stop=True, tile_position=(0, 0))
                    nc.tensor.matmul(out=pst[:, 512:1024],
                                     lhsT=k_sb[pair][64:128, msl], rhs=q_sb[pair][64:128, qsl],
                                     start=True, stop=True, tile_position=(64, 0))
                    et = pex.tile([128, 1024], BF16, tag=f"e{pair}", name=f"et{pair}_{nt}_{mc}")
                    if EXP_ON_ACT[(pair, nt)]:
                        nc.scalar.activation(et[:, :], pst[:, :], ActFn.Exp)
                    else:
                        y1 = pex.tile([128, 1024], F32, tag="y1", name=f"y1_{pair}_{nt}_{mc}")
                        emit_exp_dve(nc, exp_op, et[:, :], pst[:, :], y1[:, :], a3_t[:, :])
                    if _DBG and nt == 0 and mc == 0:
                        nc.sync.dma_start(out=dbg[f"e{pair}"][:, :], in_=et[:, :])
                        if pair == 0:
                            st_f = sb.tile([128, 1024], F32, tag="dbg_st", name="dbg_st_t")
                            nc.vector.tensor_copy(st_f, pst[:, :])
                            nc.sync.dma_start(out=dbg["st0"][:, :], in_=st_f)
                    exps.append(et)
                first, last = mc == 0, mc == N_MC - 1
                for pair in (0, 1):
                    vb = mc * 256 + pair * 128
                    nc.tensor.matmul(out=po[pair][0:64, :],
                                     lhsT=vT_sb[:, vb:vb + 64], rhs=exps[pair][:, 0:512],
                                     start=first, stop=last, tile_position=(0, 0))
                    nc.tensor.matmul(out=po[pair][64:128, :],
                                     lhsT=vT_sb[:, vb + 64:vb + 128], rhs=exps[pair][:, 512:1024],
                                     start=first, stop=last, tile_position=(0, 64))
                for hh in range(4):
                    nc.tensor.matmul(out=prs[32 * hh:32 * hh + 1, :],
                                     lhsT=ones[:, :], rhs=exps[hh // 2][:, (hh % 2) * 512:(hh % 2 + 1) * 512],
                                     start=first, stop=last, tile_position=(0, 32 * hh))
            if _DBG and nt == 0:
                rs_f = sb.tile([128, 512], F32, tag="dbg_rs", name="dbg_rs_t")
                nc.vector.tensor_copy(rs_f[0:1, :], prs[0:1, :])
                nc.vector.tensor_copy(rs_f[32:33, :], prs[32:33, :])
                nc.vector.tensor_copy(rs_f[64:65, :], prs[64:65, :])
                nc.vector.tensor_copy(rs_f[96:97, :], prs[96:97, :])
                nc.sync.dma_start(out=dbg["rs"][:, :], in_=rs_f)
                po_f = sb.tile([128, 512], F32, tag="dbg_po", name="dbg_po_t")
                nc.vector.tensor_copy(po_f, po[1][:, :])
                nc.sync.dma_start(out=dbg["po0"][:, :], in_=po_f)
            # normalization: copy the 4 rowsum rows to SBUF (ACT, f32r), matmul
            # against the 0/1 broadcast matrix to replicate each head's rowsum
            # to its 64 output partitions, evacuate to SBUF, reciprocal, multiply.
            # (custom DVE ops and partition_broadcast only work at base 0.)
            rs_sb = sb.tile([128, 512], F32R, tag="rs_sb", name=f"rs_sb{nt}")
            for hh in range(4):
                nc.scalar.copy(rs_sb[32 * hh:32 * hh + 1, :], prs[32 * hh:32 * hh + 1, :])
            for oc in range(2):
                pb = ps.tile([128, 512], F32, tag="pb", name=f"pb{oc}_{nt}")
                nc.tensor.matmul(out=pb[:, :], lhsT=bc[:, oc * 128:(oc + 1) * 128],
                                 rhs=rs_sb[:, :], start=True, stop=True)
                rbr = sb.tile([128, 512], F32, tag="rbr", name=f"rbr{oc}_{nt}")
                nc.scalar.copy(rbr[:, :], pb[:, :])
                rb = sb.tile([128, 512], F32, tag="rb", name=f"rb{oc}_{nt}")
                nc.vector.reciprocal_approx_fast(out=rb[:, :], in_=rbr[:, :])
                nc.vector.tensor_tensor(
                    out=out_sp[oc][:, qsl], in0=po[oc][:, :], in1=rb[:, :],
                    op=mybir.AluOpType.mult)
                if _DBG and nt == 0:
                    nc.sync.dma_start(out=dbg[f"rb{oc}"][:, :], in_=rb[:, :])
            if _DBG and nt == 0:
                nc.sync.dma_start(out=dbg["rsinv"][:, :], in_=rs_sb[:, :])

        # ---------------- phase 3: output projection + bias ----------------
        for oc in range(2):
            for nt in range(N_NT):
                sl = slice(nt * 512, (nt + 1) * 512)
                py = ps.tile([128, 512], F32, tag=f"o{oc}", name=f"py{oc}_{nt}")
                nc.tensor.matmul(out=py[:, :], lhsT=w_sb["wp"][0][:, oc * 128:(oc + 1) * 128],
                                 rhs=out_sp[0][:, sl], start=True, stop=False)
                nc.tensor.matmul(out=py[:, :], lhsT=w_sb["wp"][1][:, oc * 128:(oc + 1) * 128],
                                 rhs=out_sp[1][:, sl], start=False, stop=True)
                y_sb = pout.tile([128, 512], F32, tag="y", name=f"y_sb{oc}_{nt}")
                nc.vector.tensor_scalar_add(y_sb[:, :], py[:, :], bias_sb[:, oc:oc + 1])
                nc.sync.dma_start(out=y_d[oc * 128:(oc + 1) * 128, sl], in_=y_sb[:, :])

    nc.compile()
    return nc


_CACHE = {}


def _get_program():
    if "nc" not in _CACHE:
        op = register_exp_op()
        _CACHE["nc"] = build_program(op)
    return _CACHE["nc"]


def kernel(x, w_qkv, w_proj, b_proj):
    x = np.asarray(x, np.float32)
    w_qkv = np.asarray(w_qkv, np.float32)
    w_proj = np.asarray(w_proj, np.float32)
    b_proj = np.asarray(b_proj, np.float32)

    nc = _get_program()

    x2 = x.reshape(B, C, N)
    wq_t = np.ascontiguousarray((w_qkv[0:C] / 8.0).T)
    wk_t = np.ascontiguousarray(w_qkv[C:2 * C].T)
    wv_t = np.ascontiguousarray(w_qkv[2 * C:3 * C].T)
    wp_t = np.ascontiguousarray(w_proj.T)
    bias2 = np.ascontiguousarray(b_proj.reshape(2, 128).T)

    in_maps = []
    for core in range(8):
        b, half = divmod(core, 2)
        n0 = half * NQ
        x_rot = np.concatenate([x2[b][:, n0:], x2[b][:, :n0]], axis=1)
        in_maps.append({
            "x": np.ascontiguousarray(x_rot),
            "wq": wq_t, "wk": wk_t, "wv": wv_t, "wp": wp_t,
            "bias": bias2,
        })

    res = run_bass_kernel_spmd(nc, in_maps, list(range(8)))

    y = np.empty((B, C, N), np.float32)
    for core in range(8):
        b, half = divmod(core, 2)
        n0 = half * NQ
        y[b][:, n0:n0 + NQ] = res.results[core]["y"]
    return y.reshape(B, C, H, W)


# revision 17
# speedup vs baseline: 71.3999x; 71.3999x over previous
"""AttentionBlock (1x1-conv QKV + 4-head softmax attention + 1x1-conv proj)
on 8 Trainium2 NeuronCores.

Sharding: data-parallel over (batch b, query-half h) -> 8 shards. Each core
gets x rotated so its 2048 query columns are always columns 0:2048 (key order
is a permutation, which softmax-attention is invariant to), computes
qkv projections, 4-head attention for its half of the queries, and the output
projection for its [256, 2048] output slice. No collectives.

Core kernel tricks:
  - all matmuls in float32r (full-rate PE, ~1.5e-4 rel rounding)
  - S^T = K^T Q with two heads row-tiled in the PE array (K=64 each)
  - exp of scores: half the heads on the Scalar engine (exact), half via a
    fused custom DVE op ((x+c0)((x+c1)x+c2))^8 ~ C*e^x (scale cancels in
    softmax; assignment is per-(head, query-tile) so rows stay consistent)
  - attn @ V with two heads col-tiled, plus 4-way col-tiled ones-matmul
    rowsums accumulated in PSUM
  - softmax normalization via reciprocal + a tiny broadcast matmul
"""
import sys

sys.path.insert(0, '/opt/trn_rl_repo')

import numpy as np
from contextlib import ExitStack

from concourse import bass, bacc, mybir
import concourse.tile as tile
from concourse import dve_ops
from concourse.dve_ops import DveOp, OPS, CUSTOM_DVE_SPECS, _SUB_OPCODE_FOR_NAME
from concourse.dve_spec import Spec, Src0, Src1, C0, C1, C2, C3, lower, sq, _spill_c3_to_src1
from concourse.dve_uop import DveOpSpec
from concourse.bass_utils import run_bass_kernel_spmd

F32 = mybir.dt.float32
F32R = mybir.dt.float32r
BF16 = mybir.dt.bfloat16
ActFn = mybir.ActivationFunctionType

B, C, H, W = 4, 256, 64, 64
HEADS, DH = 4, 64
N = H * W            # 4096 keys
NQ = N // 2          # 2048 queries per core
NT = 512             # query tile (one PSUM bank of fp32)
N_NT = NQ // NT      # 4 query tiles
N_MC = N // 128      # 32 key chunks

# exp(x) ~ C * [q3(x) * (x^2 + b0 x + b1)]^16 over x in [-8.8, 8.4]
# (max rel err 3.3e-4; the constant C cancels in softmax normalization).
# Two DVE instructions: EXP5A computes the cubic q3, EXP5B multiplies by the
# monic quadratic and raises to the 16th power.
EXP_A = (0.00039684202121525346, 2.589769573122113e-05,
         6.891462469732395e-07, 7.771052073346383e-09)   # a0..a3
EXP_B = (-6.95331830849084, 2519.7822812996437)          # b0, b1


def _ref_exp5a(in0, in1, c0, c1, c2):
    x = in0.astype(np.float32)
    a3 = in1.astype(np.float32) if isinstance(in1, np.ndarray) else np.float32(in1)
    return (((a3 * x + np.float32(c2)) * x + np.float32(c1)) * x
            + np.float32(c0)).astype(np.float32)


def _ref_exp5b(in0, in1, c0, c1, c2):
    x = in0.astype(np.float32)
    q3 = in1.astype(np.float32)
    p = (q3 * ((x + np.float32(c0)) * x + np.float32(c1))).astype(np.float32)
    for _ in range(4):
        p = (p * p).astype(np.float32)
    return p


def _register(name, spec, rd1_en):
    row = dve_ops._CUSTOM_DVE_ROW_BASE + len(OPS)
    assert row < 0x20
    _SUB_OPCODE_FOR_NAME[name] = row
    shas = {}
    for ver in ("v3", "v4"):
        uops = lower(spec, ver=ver)
        shas[ver] = DveOpSpec(name=name, opcode=row, uops=uops, rd1_en=rd1_en).sha(ver)
    op = DveOp(name, spec, subdim=False, uops_sha=shas)
    OPS.append(op)
    CUSTOM_DVE_SPECS[name] = spec
    return op


def register_exp_op():
    if "EXP5A_ANT" in _SUB_OPCODE_FOR_NAME:
        a = next(op for op in OPS if op.name == "EXP5A_ANT")
        b = next(op for op in OPS if op.name == "EXP5B_ANT")
        return a, b
    x = Src0
    body_a = _spill_c3_to_src1(((C3 * x + C2) * x + C1) * x + C0)
    op_a = _register("EXP5A_ANT", Spec(body=body_a, reference=_ref_exp5a), True)
    body_b = sq(sq(sq(sq(Src1 * ((x + C0) * x + C1)))))
    op_b = _register("EXP5B_ANT", Spec(body=body_b, reference=_ref_exp5b), True)
    return op_a, op_b


def emit_exp_dve(nc, ops, out, in_, y1, a3_t):
    op_a, op_b = ops
    nc.vector._custom_dve(op_a, out=y1, in0=in_, in1=a3_t,
                          s0=float(EXP_A[0]), s1=float(EXP_A[1]), imm2=float(EXP_A[2]))
    return nc.vector._custom_dve(op_b, out=out, in0=in_, in1=y1,
                                 s0=float(EXP_B[0]), s1=float(EXP_B[1]))


# exp-engine split: ACT computes pair-0 tiles fully plus the first EXP_N0
# query-columns of each pair-1 head; the DVE two-op pipeline takes the rest.
# Constant per (pair, nt, n-range) so every softmax row uses one implementation.
import os as _os
EXP_N0 = int(_os.environ.get("EXP_N0", "192"))


def build_program(exp_op):
    nc = bacc.Bacc(target_bir_lowering=False)

    x_d = nc.declare_dram_parameter("x", [C, N], F32, isOutput=False)
    wq_d = nc.declare_dram_parameter("wq", [C, C], F32, isOutput=False)
    wk_d = nc.declare_dram_parameter("wk", [C, C], F32, isOutput=False)
    wv_d = nc.declare_dram_parameter("wv", [C, C], F32, isOutput=False)
    wp_d = nc.declare_dram_parameter("wp", [C, C], F32, isOutput=False)
    bias_d = nc.declare_dram_parameter("bias", [128, 2], F32, isOutput=False)
    y_d = nc.declare_dram_parameter("y", [C, NQ], F32, isOutput=True)
    import os as _os
    _DBG = bool(int(_os.environ.get("KERNEL_DEBUG", "0")))
    dbg = {}
    if _DBG:
        BF16_ = mybir.dt.bfloat16
        dbg["q0"] = nc.declare_dram_parameter("dbg_q0", [128, 512], F32, isOutput=True)
        dbg["st0"] = nc.declare_dram_parameter("dbg_st0", [128, 1024], F32, isOutput=True)
        dbg["e0"] = nc.declare_dram_parameter("dbg_e0", [128, 1024], BF16_, isOutput=True)
        dbg["e1"] = nc.declare_dram_parameter("dbg_e1", [128, 1024], BF16_, isOutput=True)
        dbg["rs"] = nc.declare_dram_parameter("dbg_rs", [128, 512], F32, isOutput=True)
        dbg["rsinv"] = nc.declare_dram_parameter("dbg_rsinv", [128, 512], F32, isOutput=True)
        dbg["rb0"] = nc.declare_dram_parameter("dbg_rb0", [128, 512], F32, isOutput=True)
        dbg["rb1"] = nc.declare_dram_parameter("dbg_rb1", [128, 512], F32, isOutput=True)
        dbg["po0"] = nc.declare_dram_parameter("dbg_po0", [128, 512], F32, isOutput=True)
        dbg["vt0"] = nc.declare_dram_parameter("dbg_vt0", [128, 256], BF16_, isOutput=True)

    with tile.TileContext(nc) as tc, ExitStack() as ctx:
        sb = ctx.enter_context(tc.tile_pool(name="sb", bufs=1))
        pex = ctx.enter_context(tc.tile_pool(name="pex", bufs=3))
        pout = ctx.enter_context(tc.tile_pool(name="pout", bufs=2))
        ps = ctx.enter_context(tc.tile_pool(name="ps", bufs=1, space="PSUM"))

        # ---------------- load + round inputs to f32r ----------------
        x_f = [sb.tile([128, N], F32, tag=f"xf{i}", name=f"xf{i}") for i in range(2)]
        x_r = [sb.tile([128, N], F32R, tag=f"xr{i}", name=f"xr{i}") for i in range(2)]
        for kc in range(2):
            nc.sync.dma_start(out=x_f[kc], in_=x_d[kc * 128:(kc + 1) * 128, :])
        nc.scalar.copy(x_r[0][:, :], x_f[0][:, :])
        nc.vector.tensor_copy(x_r[1][:, :], x_f[1][:, :])

        w_sb = {}
        for name, dram in (("wq", wq_d), ("wk", wk_d), ("wv", wv_d), ("wp", wp_d)):
            tiles = []
            for kc in range(2):
                f = sb.tile([128, C], F32, tag="wf", name=f"{name}f{kc}")
                nc.sync.dma_start(out=f, in_=dram[kc * 128:(kc + 1) * 128, :])
                r = sb.tile([128, C], F32R, tag=f"{name}{kc}", name=f"{name}r{kc}")
                (nc.vector.tensor_copy if kc else nc.scalar.copy)(r[:, :], f[:, :])
                tiles.append(r)
            w_sb[name] = tiles
        bias_sb = sb.tile([128, 2], F32, tag="bias")
        nc.sync.dma_start(out=bias_sb, in_=bias_d[:, :])

        # constants: ones column + broadcast matrices
        ones = sb.tile([128, 1], BF16, tag="ones")
        nc.vector.memset(ones, 1.0)
        a3_t = sb.tile([128, 1], F32, tag="a3")
        nc.vector.memset(a3_t, float(EXP_A[3]))
        zero_f = sb.tile([128, 512], F32, tag="zerof")
        nc.vector.memset(zero_f, 0.0)
        # broadcast matrix: out[m, n] = rhs[32*head(m) + 64*oc, n]
        bc_f = sb.tile([128, 256], F32, tag="bc_f")
        nc.vector.memset(bc_f, 0.0)
        nc.vector.memset(bc_f[0:1, 0:64], 1.0)
        nc.vector.memset(bc_f[32:33, 64:128], 1.0)
        nc.vector.memset(bc_f[64:65, 128:192], 1.0)
        nc.vector.memset(bc_f[96:97, 192:256], 1.0)
        bc = sb.tile([128, 256], F32R, tag="bc")
        nc.vector.tensor_copy(bc, bc_f[:, :])


        # ---------------- phase 1: qkv projections ----------------
        q_sb = [sb.tile([128, NQ], F32R, tag=f"q{oc}", name=f"q_sb{oc}") for oc in range(2)]
        k_sb = [sb.tile([128, N], F32R, tag=f"k{oc}", name=f"k_sb{oc}") for oc in range(2)]
        vT_sb = sb.tile([128, N_MC * 256], BF16, tag="vT")

        for oc in range(2):
            for nt in range(N_NT):
                pq = ps.tile([128, 512], F32, tag="s", bufs=2, name=f"pq{oc}_{nt}")
                sl = slice(nt * 512, (nt + 1) * 512)
                nc.tensor.matmul(out=pq[:, :], lhsT=w_sb["wq"][0][:, oc * 128:(oc + 1) * 128],
                                 rhs=x_r[0][:, sl], start=True, stop=False)
                nc.tensor.matmul(out=pq[:, :], lhsT=w_sb["wq"][1][:, oc * 128:(oc + 1) * 128],
                                 rhs=x_r[1][:, sl], start=False, stop=True)
                (nc.scalar.copy if (oc + nt) % 2 else nc.vector.tensor_copy)(q_sb[oc][:, sl], pq[:, :])
                if _DBG and oc == 0 and nt == 0:
                    nc.sync.dma_start(out=dbg["q0"][:, :], in_=q_sb[0][:, 0:512].bitcast(F32))
        for oc in range(2):
            for nt in range(2 * N_NT):
                pk = ps.tile([128, 512], F32, tag="s", bufs=2, name=f"pk{oc}_{nt}")
                sl = slice(nt * 512, (nt + 1) * 512)
                nc.tensor.matmul(out=pk[:, :], lhsT=w_sb["wk"][0][:, oc * 128:(oc + 1) * 128],
                                 rhs=x_r[0][:, sl], start=True, stop=False)
                nc.tensor.matmul(out=pk[:, :], lhsT=w_sb["wk"][1][:, oc * 128:(oc + 1) * 128],
                                 rhs=x_r[1][:, sl], start=False, stop=True)
                (nc.vector.tensor_copy if nt % 2 else nc.scalar.copy)(k_sb[oc][:, sl], pk[:, :])
        for mc in range(N_MC):
            pv = ps.tile([128, 256], F32, tag="rs", name=f"pv{mc}")
            msl = slice(mc * 128, (mc + 1) * 128)
            nc.tensor.matmul(out=pv[:, :], lhsT=x_r[0][:, msl], rhs=w_sb["wv"][0][:, :],
                             start=True, stop=False)
            nc.tensor.matmul(out=pv[:, :], lhsT=x_r[1][:, msl], rhs=w_sb["wv"][1][:, :],
                             start=False, stop=True)
            (nc.vector.tensor_copy if mc % 2 else nc.scalar.copy)(
                vT_sb[:, mc * 256:(mc + 1) * 256], pv[:, :])
            if _DBG and mc == 0:
                nc.sync.dma_start(out=dbg["vt0"][:, :], in_=vT_sb[:, 0:256])

        import os
        _PH = int(os.environ.get("KERNEL_PHASES", "3"))
        # ---------------- phase 2: attention ----------------
        out_sp = [sb.tile([128, NQ], F32R, tag=f"osp{oc}", name=f"out_sp{oc}") for oc in range(2)]

        if _PH < 2:
            zero_f = sb.tile([128, 512], F32, tag="zero_f")
            nc.vector.memset(zero_f, 0.0)
            for oc in range(2):
                for z in range(4):
                    nc.vector.tensor_copy(out_sp[oc][:, z * 512:(z + 1) * 512], zero_f[:, :])
        for nt in range(N_NT if _PH >= 2 else 0):
            qsl = slice(nt * 512, (nt + 1) * 512)
            po = [ps.tile([128, 512], F32, tag="o", bufs=3, name=f"po{pair}_{nt}") for pair in (0, 1)]
            prs = ps.tile([128, 512], F32, tag="rs", name=f"prs{nt}")
            for mc in range(N_MC):
                msl = slice(mc * 128, (mc + 1) * 128)
                exps = []
                for pair in (0, 1):
                    pst = ps.tile([128, 1024], F32, tag="s", bufs=2, name=f"pst{pair}_{nt}_{mc}")
                    # S^T: two heads row-tiled (dh=64 each)
                    nc.tensor.matmul(out=pst[:, 0:512],
                                     lhsT=k_sb[pair][0:64, msl], rhs=q_sb[pair][0:64, qsl],
                                     start=True, stop=True, tile_position=(0, 0))
                    nc.tensor.matmul(out=pst[:, 512:1024],
                                     lhsT=k_sb[pair][64:128, msl], rhs=q_sb[pair][64:128, qsl],
                                     start=True, stop=True, tile_position=(64, 0))
                    et = pex.tile([128, 1024], BF16, tag=f"e{pair}", name=f"et{pair}_{nt}_{mc}")
                    if pair == 0 or EXP_N0 >= 512:
                        nc.scalar.activation(et[:, :], pst[:, :], ActFn.Exp)
                    elif EXP_N0 == 0:
                        y1 = pex.tile([128, 1024], F32, tag="y1", name=f"y1_{pair}_{nt}_{mc}")
                        emit_exp_dve(nc, exp_op, et[:, :], pst[:, :], y1[:, :], a3_t[:, :])
                    else:
                        # strided APs covering (h2 cols [a:b]) u (h3 cols [512+a:512+b])
                        def _two(ap_t, a, b):
                            base = ap_t[:, a:b]
                            return bass.AP(tensor=base.tensor, offset=base.offset,
                                           ap=[list(base.ap[0]), [512, 2], [1, b - a]])
                        nc.scalar.activation(_two(et, 0, EXP_N0), _two(pst, 0, EXP_N0),
                                             ActFn.Exp)
                        y1 = pex.tile([128, 1024], F32, tag="y1", name=f"y1_{pair}_{nt}_{mc}")
                        emit_exp_dve(nc, exp_op, _two(et, EXP_N0, 512),
                                     _two(pst, EXP_N0, 512), _two(y1, EXP_N0, 512),
                                     a3_t[:, :])
                    if _DBG and nt == 0 and mc == 0:
                        nc.sync.dma_start(out=dbg[f"e{pair}"][:, :], in_=et[:, :])
                        if pair == 0:
                            st_f = sb.tile([128, 1024], F32, tag="dbg_st", name="dbg_st_t")
                            nc.vector.tensor_copy(st_f, pst[:, :])
                            nc.sync.dma_start(out=dbg["st0"][:, :], in_=st_f)
                    exps.append(et)
                first, last = mc == 0, mc == N_MC - 1
                for pair in (0, 1):
                    vb = mc * 256 + pair * 128
                    nc.tensor.matmul(out=po[pair][0:64, :],
                                     lhsT=vT_sb[:, vb:vb + 64], rhs=exps[pair][:, 0:512],
                                     start=first, stop=last, tile_position=(0, 0))
                    nc.tensor.matmul(out=po[pair][64:128, :],
                                     lhsT=vT_sb[:, vb + 64:vb + 128], rhs=exps[pair][:, 512:1024],
                                     start=first, stop=last, tile_position=(0, 64))
                for hh in range(4):
                    nc.tensor.matmul(out=prs[32 * hh:32 * hh + 1, :],
                                     lhsT=ones[:, :], rhs=exps[hh // 2][:, (hh % 2) * 512:(hh % 2 + 1) * 512],
                                     start=first, stop=last, tile_position=(0, 32 * hh))
            if _DBG and nt == 0:
                rs_f = sb.tile([128, 512], F32, tag="dbg_rs", name="dbg_rs_t")
                nc.vector.tensor_copy(rs_f[0:1, :], prs[0:1, :])
                nc.vector.tensor_copy(rs_f[32:33, :], prs[32:33, :])
                nc.vector.tensor_copy(rs_f[64:65, :], prs[64:65, :])
                nc.vector.tensor_copy(rs_f[96:97, :], prs[96:97, :])
                nc.sync.dma_start(out=dbg["rs"][:, :], in_=rs_f)
                po_f = sb.tile([128, 512], F32, tag="dbg_po", name="dbg_po_t")
                nc.vector.tensor_copy(po_f, po[1][:, :])
                nc.sync.dma_start(out=dbg["po0"][:, :], in_=po_f)
            # normalization: copy the 4 rowsum rows to SBUF (ACT, f32r), matmul
            # against the 0/1 broadcast matrix to replicate each head's rowsum
            # to its 64 output partitions, evacuate to SBUF, reciprocal, multiply.
            # (custom DVE ops and partition_broadcast only work at base 0.)
            rs_sb = sb.tile([128, 512], F32R, tag="rs_sb", name=f"rs_sb{nt}")
            # zero-fill: the broadcast matmul reads all 128 partitions and
            # uninitialized SBUF can contain NaNs (0 * NaN = NaN)
            nc.vector.tensor_copy(rs_sb[:, :], zero_f[:, :])
            for hh in range(4):
                nc.scalar.copy(rs_sb[32 * hh:32 * hh + 1, :], prs[32 * hh:32 * hh + 1, :])
            for oc in range(2):
                pb = ps.tile([128, 512], F32, tag="s", bufs=2, name=f"pb{oc}_{nt}")
                nc.tensor.matmul(out=pb[:, :], lhsT=bc[:, oc * 128:(oc + 1) * 128],
                                 rhs=rs_sb[:, :], start=True, stop=True)
                rbr = sb.tile([128, 512], F32, tag="rbr", name=f"rbr{oc}_{nt}")
                nc.scalar.copy(rbr[:, :], pb[:, :])
                rb = sb.tile([128, 512], F32, tag="rb", name=f"rb{oc}_{nt}")
                nc.vector.reciprocal_approx_fast(out=rb[:, :], in_=rbr[:, :])
                nc.vector.tensor_tensor(
                    out=out_sp[oc][:, qsl], in0=po[oc][:, :], in1=rb[:, :],
                    op=mybir.AluOpType.mult)
                if _DBG and nt == 0:
                    nc.sync.dma_start(out=dbg[f"rb{oc}"][:, :], in_=rb[:, :])
            if _DBG and nt == 0:
                nc.sync.dma_start(out=dbg["rsinv"][:, :], in_=rs_sb[:, :])

        # ---------------- phase 3: output projection + bias ----------------
        for oc in range(2):
            for nt in range(N_NT):
                sl = slice(nt * 512, (nt + 1) * 512)
                py = ps.tile([128, 512], F32, tag="o", bufs=3, name=f"py{oc}_{nt}")
                nc.tensor.matmul(out=py[:, :], lhsT=w_sb["wp"][0][:, oc * 128:(oc + 1) * 128],
                                 rhs=out_sp[0][:, sl], start=True, stop=False)
                nc.tensor.matmul(out=py[:, :], lhsT=w_sb["wp"][1][:, oc * 128:(oc + 1) * 128],
                                 rhs=out_sp[1][:, sl], start=False, stop=True)
                y_sb = pout.tile([128, 512], F32, tag="y", name=f"y_sb{oc}_{nt}")
                nc.vector.tensor_scalar_add(y_sb[:, :], py[:, :], bias_sb[:, oc:oc + 1])
                nc.sync.dma_start(out=y_d[oc * 128:(oc + 1) * 128, sl], in_=y_sb[:, :])

    nc.compile()
    return nc


_CACHE = {}


def _get_program():
    if "nc" not in _CACHE:
        op = register_exp_op()
        _CACHE["nc"] = build_program(op)
    return _CACHE["nc"]


def kernel(x, w_qkv, w_proj, b_proj):
    x = np.asarray(x, np.float32)
    w_qkv = np.asarray(w_qkv, np.float32)
    w_proj = np.asarray(w_proj, np.float32)
    b_proj = np.asarray(b_proj, np.float32)

    nc = _get_program()

    x2 = x.reshape(B, C, N)
    wq_t = np.ascontiguousarray((w_qkv[0:C] / 8.0).T)
    wk_t = np.ascontiguousarray(w_qkv[C:2 * C].T)
    wv_t = np.ascontiguousarray(w_qkv[2 * C:3 * C].T)
    wp_t = np.ascontiguousarray(w_proj.T)
    bias2 = np.ascontiguousarray(b_proj.reshape(2, 128).T)

    in_maps = []
    for core in range(8):
        b, half = divmod(core, 2)
        n0 = half * NQ
        x_rot = np.concatenate([x2[b][:, n0:], x2[b][:, :n0]], axis=1)
        in_maps.append({
            "x": np.ascontiguousarray(x_rot),
            "wq": wq_t, "wk": wk_t, "wv": wv_t, "wp": wp_t,
            "bias": bias2,
        })

    res = run_bass_kernel_spmd(nc, in_maps, list(range(8)))

    y = np.empty((B, C, N), np.float32)
    for core in range(8):
        b, half = divmod(core, 2)
        n0 = half * NQ
        y[b][:, n0:n0 + NQ] = res.results[core]["y"]
    return y.reshape(B, C, H, W)
